# revision 3
# baseline (speedup 1.0000x reference)
"""Trainium2 Bass kernel for the 4-layer LIF spiking network (EventDrivenSparseNetwork).

Strategy:
  - Data-parallel over batch: B=32 sharded 4-per-core across 8 NeuronCores,
    weights replicated. No cross-core communication.
  - Per core and layer: dense fp32 GEMM cur = spikes @ (W*mask)^T on TensorE
    (fp32 for numerical fidelity -- the spiking threshold makes the network
    chaotic, reduced-precision matmuls flip spikes), then the LIF scan on
    VectorE: v = alpha*v + cur_t; s = (v >= 1); v = (v < 1)*v.
  - Activations live in SBUF as [128p, 16no, 400f] with n = no*128+p and
    f = t*4 + b so the scan's per-timestep slice is a cheap strided AP and a
    layer's spikes feed the next layer's GEMM with no reshuffling.
  - T is processed in chunks so the scan of chunk c overlaps the GEMM of
    chunk c+1 (and the next layer's first chunk), keeping TensorE dense.
  - All transposes / masking / sharding happen host-side in numpy; the
    device sees contiguous DMA-friendly layouts.
"""

import sys

sys.path.insert(0, "/opt/trn_rl_repo")

import numpy as np

B, T, N = 32, 100, 2048
NL = 4
NCORES = 8
BL = B // NCORES          # 4 samples per core
NO = N // 128             # 16 output-neuron chunks
KO = N // 128             # 16 contraction chunks
F = T * BL                # 400, f = t*BL + b
ALPHA = float(np.float32(np.exp(np.float32(-1.0 / 20.0))))
CHUNKS = 2
FC = F // CHUNKS
TC = T // CHUNKS


def build(reps: int = 1, chunks: int = CHUNKS, internal_weights: bool = False):
    """Build (and bacc-compile) the SPMD kernel. Returns the Bass object."""
    from concourse import mybir, bacc
    import concourse.tile as tile

    fc = F // chunks
    tc_steps = T // chunks

    nc = bacc.Bacc("TRN2", target_bir_lowering=False, debug=False,
                   num_devices=NCORES)
    wkind = {} if internal_weights else {"kind": "ExternalInput"}
    w_d = nc.dram_tensor("w", [NL, NO, 128, KO, 128], mybir.dt.float32,
                         **wkind).ap()
    x_d = nc.dram_tensor("x", [128, KO, F], mybir.dt.float32,
                         kind="ExternalInput").ap()
    out_d = nc.dram_tensor("out", [128, NO, F], mybir.dt.float32,
                           kind="ExternalOutput").ap()

    import contextlib
    with tile.TileContext(nc) as tctx:
        with contextlib.ExitStack() as stack:
            actsp = stack.enter_context(tctx.tile_pool(name="acts", bufs=3))
            wp = stack.enter_context(tctx.tile_pool(name="wp", bufs=6))
            curp = stack.enter_context(tctx.tile_pool(name="curp", bufs=2))
            vp = stack.enter_context(tctx.tile_pool(name="vp", bufs=2))
            pp = stack.enter_context(tctx.tile_pool(name="pp", bufs=6,
                                                    space="PSUM"))

            def body(_iv=None):
                acts = actsp.tile([128, KO, F], mybir.dt.float32, tag="acts")
                nc.sync.dma_start(acts[:, :, :], x_d[:, :, :])
                cur_in = acts
                for l in range(NL):
                    spk = actsp.tile([128, NO, F], mybir.dt.float32,
                                     tag="acts")
                    vt = vp.tile([128, NO, BL], mybir.dt.float32, tag="v")
                    nc.vector.memset(vt[:, :, :], 0.0)
                    for c in range(chunks):
                        f0 = c * fc
                        cur = curp.tile([128, NO, fc], mybir.dt.float32,
                                        tag="cur")
                        for mo in range(NO):
                            wt = wp.tile([128, KO, 128], mybir.dt.float32,
                                         tag="w")
                            nc.sync.dma_start(wt[:, :, :], w_d[l, mo])
                            pt = pp.tile([128, fc], mybir.dt.float32)
                            for ko in range(KO):
                                nc.tensor.matmul(
                                    pt[:, :], wt[:, ko, :],
                                    cur_in[:, ko, f0:f0 + fc],
                                    start=(ko == 0), stop=(ko == KO - 1))
                            nc.scalar.copy(cur[:, mo, :], pt[:, :])
                        for ts in range(tc_steps):
                            tl = slice(ts * BL, (ts + 1) * BL)
                            gl = slice(f0 + ts * BL, f0 + (ts + 1) * BL)
                            nc.vector.scalar_tensor_tensor(
                                vt[:, :, :], vt[:, :, :], ALPHA,
                                cur[:, :, tl],
                                op0=mybir.AluOpType.mult,
                                op1=mybir.AluOpType.add)
                            nc.vector.tensor_scalar(
                                spk[:, :, gl], vt[:, :, :], 1.0, None,
                                op0=mybir.AluOpType.is_ge)
                            nc.vector.scalar_tensor_tensor(
                                vt[:, :, :], vt[:, :, :], 1.0, vt[:, :, :],
                                op0=mybir.AluOpType.is_lt,
                                op1=mybir.AluOpType.mult)
                    cur_in = spk
                nc.sync.dma_start(out_d[:, :, :], cur_in[:, :, :])

            if reps == 1:
                body()
            else:
                with tctx.For_i(0, reps, 1) as iv:
                    body(iv)
    nc.compile()
    return nc


def prep_weights(inputs):
    """[NL, NO, 128p, KO, 128mi] fp32: w[l, mo, p, ko, mi] = Wm_l[mo*128+mi, ko*128+p]."""
    w = np.empty((NL, NO, 128, KO, 128), np.float32)
    for l in range(NL):
        wm = (np.asarray(inputs[f"W{l}"], np.float32)
              * np.asarray(inputs[f"mask{l}"]).astype(np.float32))
        # wmT[n, m] with n contraction; chunk to [ko, p, mo, mi] -> [mo, p, ko, mi]
        wmT = np.ascontiguousarray(wm.T)
        w[l] = (wmT.reshape(KO, 128, NO, 128)
                .transpose(2, 1, 0, 3))  # [mo, p, ko, mi]
    return w


def prep_x(x_core):
    """x_core [BL, T, N] -> [128, KO, F] with f = t*BL+b, n = no*128+p."""
    xt = x_core.transpose(2, 1, 0)                 # [n, t, b]
    xt = xt.reshape(KO, 128, T, BL).transpose(1, 0, 2, 3)  # [p, no, t, b]
    return np.ascontiguousarray(xt.reshape(128, KO, F), dtype=np.float32)


def unprep_out(o):
    """[128, NO, F] -> [BL, T, N]."""
    o = o.reshape(128, NO, T, BL).transpose(1, 0, 2, 3)    # [no, p, t, b]
    o = o.reshape(N, T, BL).transpose(2, 1, 0)             # [b, t, n]
    return np.ascontiguousarray(o)


_cached_nc = None


def kernel(**inputs) -> np.ndarray:
    global _cached_nc
    from concourse.bass_utils import run_bass_kernel_spmd

    if _cached_nc is None:
        _cached_nc = build(reps=1)
    nc = _cached_nc

    w = prep_weights(inputs)
    x = np.asarray(inputs["x"], np.float32)
    in_maps = [{"w": w, "x": prep_x(x[ci * BL:(ci + 1) * BL])}
               for ci in range(NCORES)]
    res = run_bass_kernel_spmd(nc, in_maps, core_ids=list(range(NCORES)))
    out = np.empty((B, T, N), np.float32)
    for ci in range(NCORES):
        out[ci * BL:(ci + 1) * BL] = unprep_out(res.results[ci]["out"])
    return out
